# revision 1
# baseline (speedup 1.0000x reference)
"""Chamfer loss kernel for Trainium2 (8 NeuronCores, Bass/Tile).

Problem: x (4, 8192, 3), y (4, 8192, 3) fp32.
  dist[b,i,j] = ||x_bi||^2 + ||y_bj||^2 - 2 x_bi . y_bj
  out = mean_b( mean_i min_j dist + mean_j min_i dist )

Sharding: 8 cores = 4 batches x 2 halves. Core (b, h) computes
  - x->y mins for x rows [h*4096, (h+1)*4096) of batch b vs ALL y[b]
  - y->x mins for y rows [h*4096, (h+1)*4096) of batch b vs ALL x[b]
so no cross-core reduction is needed (each core owns full rows of output).

On-chip compute: dist[i,j] = ||q_i||^2 - 2 q_i . d_j + ||d_j||^2 via a single
K=24 bf16 matmul using 3-term hi/mid/lo splitting (beyond-fp32 accuracy at
bf16 matmul speed; matmul cost depends only on the free dim, not K):
  q = A + AL + AL2 (+ 2^-27),  -2d = C + E + E2,  ||d||^2 = d2h + d2l + d2l2,
  ||q||^2 = q2h + q2l + q2l2 (paired against a ones row)
Putting the full distance (not just the d-dependent part) in PSUM keeps the
interesting values near 0, which makes float16 rounding of them harmless
(relative error 2^-11 on the min; the mean over 65536 rows averages it out).
Then min_j dist = ||q_i||^2 + min_j G[i,j]; the min runs on VectorE from PSUM.
The ||q_i||^2 add + means happen on the host in float64 (cheap: O(N)).

Drain pipeline (the bottleneck: every PSUM element crosses DVE@0.96GHz or
ACT@1.2GHz once): PSUM groups of 1024 fp32 (2 banks) are processed in pairs:
ScalarE copies group 2k to SBUF, VectorE runs tensor_tensor_scan
(state = min(min(psum_grp_{2k+1}[t], state), sbuf_copy[t])) whose final
column is the running min of BOTH groups; the scan state chains across pairs
via `initial`, so one block (8192 db points) ends as a single [128,1] column.
This halves DVE element traffic vs per-group tensor_reduce.
(tensor_tensor_reduce would fuse the same thing but is broken on HW here.)
"""

import numpy as np
import ml_dtypes

B = 4
N = 8192  # x points per batch
M = 8192  # y points per batch
D = 3
NCORES = 8

QROWS = 4096  # query rows per core (half of a batch's points)
DBN = 8192  # database points scanned per query
KDIM = 24  # augmented contraction dim
BLKP = 128  # query rows per matmul block (PSUM partitions)
FREE = 512  # matmul free size (one PSUM fp32 bank)
GROUP = 1024  # PSUM group per drain op (2 banks)
NPAIR = DBN // (2 * GROUP)  # group-pairs per block -> output cols per block

_NC_CACHE = {}


def _build_nc(qrows=QROWS, dbn=DBN, repeat=1, drain="hybrid"):
    """Build + compile the (SPMD, identical on all cores) Bass program.

    repeat>1 wraps the compute body in a For_i loop (identical iterations) —
    used only for slope-timing the kernel on hardware.
    drain:
      "hybrid" - ACT converts most PSUM groups to fp16 in SBUF; DVE mins them
                 with 2x-rate fp16 tensor_tensor ops + direct-reduces the rest
      "scan"   - ACT copy + DVE pair-scan (HW: scan recurrence is 1x, slow)
      "reduce" - DVE tensor_reduce only, 4-bank groups
      "none"   - matmuls only (PE roofline probe)
    """
    from contextlib import ExitStack

    import concourse.tile as tile
    from concourse import bacc, mybir

    bf16 = mybir.dt.bfloat16
    f32 = mybir.dt.float32

    nblk = qrows // BLKP
    npair = dbn // (2 * GROUP)
    outc = 2 * nblk if drain == "hybrid" else nblk

    nc = bacc.Bacc(
        "TRN2", target_bir_lowering=False, debug=False, num_devices=NCORES
    )
    lx = nc.dram_tensor("lx", [KDIM, qrows], bf16, kind="ExternalInput")
    ry = nc.dram_tensor("ry", [KDIM, dbn], bf16, kind="ExternalInput")
    ly = nc.dram_tensor("ly", [KDIM, qrows], bf16, kind="ExternalInput")
    rx = nc.dram_tensor("rx", [KDIM, dbn], bf16, kind="ExternalInput")
    ox = nc.dram_tensor("ox", [BLKP, outc], f32, kind="ExternalOutput")
    oy = nc.dram_tensor("oy", [BLKP, outc], f32, kind="ExternalOutput")

    with tile.TileContext(nc) as tc, ExitStack() as ctx:
        cpool = ctx.enter_context(tc.tile_pool(name="consts", bufs=1))
        psum_bufs = 16384 // (GROUP * 4)  # fill all 8 PSUM banks
        if drain in ("reduce", "hybrid"):
            psum_bufs = 2  # [128, 2048] 4-bank groups
        ppool = ctx.enter_context(
            tc.tile_pool(name="psum", bufs=psum_bufs, space="PSUM")
        )
        spool = ctx.enter_context(tc.tile_pool(name="scratch", bufs=3))
        wpool = ctx.enter_context(tc.tile_pool(name="waste", bufs=2))
        opool = ctx.enter_context(tc.tile_pool(name="outs", bufs=1))

        s_lx = cpool.tile([KDIM, qrows], bf16, tag="lx")
        s_ry = cpool.tile([KDIM, dbn], bf16, tag="ry")
        s_ly = cpool.tile([KDIM, qrows], bf16, tag="ly")
        s_rx = cpool.tile([KDIM, dbn], bf16, tag="rx")
        nc.sync.dma_start(s_lx[:], lx[:])
        nc.sync.dma_start(s_ry[:], ry[:])
        nc.sync.dma_start(s_ly[:], ly[:])
        nc.sync.dma_start(s_rx[:], rx[:])

        s_ox = opool.tile([BLKP, outc], f32, tag="ox")
        s_oy = opool.tile([BLKP, outc], f32, tag="oy")

        def fill_group(lhs_blk, s_r, g):
            """Emit matmuls computing G for db cols [g*GROUP, (g+1)*GROUP)."""
            ps = ppool.tile([BLKP, GROUP], f32, tag="ps")
            for t in range(GROUP // FREE):
                col0 = g * GROUP + t * FREE
                nc.tensor.matmul(
                    ps[:, t * FREE : (t + 1) * FREE],
                    lhs_blk,
                    s_r[:, col0 : col0 + FREE],
                    start=True,
                    stop=True,
                )
            return ps

        if drain == "none":
            nc.gpsimd.memset(s_ox[:], 0.0)
            nc.gpsimd.memset(s_oy[:], 0.0)
        elif drain == "hybrid":
            big = float(np.finfo(np.float32).max)
            nc.gpsimd.memset(s_ox[:], big)
            nc.gpsimd.memset(s_oy[:], big)

        loop_ctx = tc.For_i(0, repeat, 1) if repeat > 1 else None
        if loop_ctx is not None:
            ctx.enter_context(loop_ctx)

        for s_l, s_r, s_o, o_dram in (
            (s_lx, s_ry, s_ox, ox),
            (s_ly, s_rx, s_oy, oy),
        ):
            for blk in range(nblk):
                lhs_blk = s_l[:, blk * BLKP : (blk + 1) * BLKP]
                if drain == "hybrid":
                    G2 = 2048
                    ngroups = dbn // G2
                    f16 = mybir.dt.float16
                    min_op = mybir.AluOpType.min

                    def fill2(g):
                        ps = ppool.tile([BLKP, G2], f32, tag="ps2")
                        for t in range(G2 // FREE):
                            col0 = g * G2 + t * FREE
                            nc.tensor.matmul(
                                ps[:, t * FREE : (t + 1) * FREE],
                                lhs_blk,
                                s_r[:, col0 : col0 + FREE],
                                start=True,
                                stop=True,
                            )
                        return ps

                    def tree16(S, width, col):
                        """TT16-halve S[:, :width] down to 1024, then reduce."""
                        cur, w = S, width
                        while w > 1024:
                            nxt = spool.tile(
                                [BLKP, w // 2], f16, tag=f"t{w // 2}"
                            )
                            nc.vector.tensor_tensor(
                                nxt[:],
                                cur[:, 0 : w // 2],
                                cur[:, w // 2 : w],
                                op=min_op,
                            )
                            cur, w = nxt, w // 2
                        nc.vector.tensor_reduce(
                            s_o[:, col : col + 1],
                            cur[:],
                            axis=mybir.AxisListType.X,
                            op=min_op,
                        )

                    direct = blk % 2 == 1 and ngroups == 4
                    g0 = 0
                    if direct:
                        ps = fill2(0)
                        nc.vector.tensor_reduce(
                            s_o[:, 2 * blk : 2 * blk + 1],
                            ps[:],
                            axis=mybir.AxisListType.X,
                            op=min_op,
                        )
                        g0 = 1
                    na = ngroups - g0
                    S = spool.tile([BLKP, na * G2], f16, tag=f"s16_{na}")
                    for g in range(g0, ngroups):
                        ps = fill2(g)
                        o0 = (g - g0) * G2
                        nc.scalar.copy(S[:, o0 : o0 + G2], ps[:])
                    if na == 3:
                        # 6144 wide: fold the odd group in with two TTs
                        T1 = spool.tile([BLKP, G2], f16, tag="t6a")
                        nc.vector.tensor_tensor(
                            T1[:], S[:, 0:G2], S[:, G2 : 2 * G2], op=min_op
                        )
                        T2 = spool.tile([BLKP, G2], f16, tag="t6b")
                        nc.vector.tensor_tensor(
                            T2[:], T1[:], S[:, 2 * G2 : 3 * G2], op=min_op
                        )
                        tree16(T2, G2, 2 * blk + 1)
                    else:
                        tree16(S, na * G2, 2 * blk + (1 if direct else 0))
                elif drain == "scan":
                    prev = None
                    for p in range(npair):
                        ps_a = fill_group(lhs_blk, s_r, 2 * p)
                        sb_a = spool.tile([BLKP, GROUP], f32, tag="sb")
                        nc.scalar.copy(sb_a[:], ps_a[:])
                        ps_b = fill_group(lhs_blk, s_r, 2 * p + 1)
                        waste = wpool.tile([BLKP, GROUP], f32, tag="w")
                        init = (
                            float(np.finfo(np.float32).max)
                            if prev is None
                            else prev[:, GROUP - 1 : GROUP]
                        )
                        nc.vector.tensor_tensor_scan(
                            waste[:],
                            ps_b[:],
                            sb_a[:],
                            initial=init,
                            op0=mybir.AluOpType.min,
                            op1=mybir.AluOpType.min,
                        )
                        prev = waste
                    nc.vector.tensor_copy(
                        s_o[:, blk : blk + 1], prev[:, GROUP - 1 : GROUP]
                    )
                elif drain == "reduce":
                    # [128, 2048] 4-bank groups, one DVE reduce each, partial
                    # mins land in s_o columns (blk*2+g of 2*nblk <= outc)
                    for g in range(2):
                        ps = ppool.tile([BLKP, 2048], f32, tag="ps2")
                        for t in range(2048 // FREE):
                            col0 = g * 2048 + t * FREE
                            nc.tensor.matmul(
                                ps[:, t * FREE : (t + 1) * FREE],
                                lhs_blk,
                                s_r[:, col0 : col0 + FREE],
                                start=True,
                                stop=True,
                            )
                        # dbn=8192 needs 4 groups; fold 2 into same col via 2 reduces
                        for gg in range(2):
                            pass
                        nc.vector.tensor_reduce(
                            s_o[:, blk : blk + 1],
                            ps[:],
                            axis=mybir.AxisListType.X,
                            op=mybir.AluOpType.min,
                        )
                    for g in range(2):
                        ps = ppool.tile([BLKP, 2048], f32, tag="ps2")
                        for t in range(2048 // FREE):
                            col0 = 4096 + g * 2048 + t * FREE
                            nc.tensor.matmul(
                                ps[:, t * FREE : (t + 1) * FREE],
                                lhs_blk,
                                s_r[:, col0 : col0 + FREE],
                                start=True,
                                stop=True,
                            )
                        nc.vector.tensor_reduce(
                            s_o[:, blk : blk + 1],
                            ps[:],
                            axis=mybir.AxisListType.X,
                            op=mybir.AluOpType.min,
                        )
                else:  # drain == "none": matmuls only
                    for g in range(npair * 2):
                        fill_group(lhs_blk, s_r, g)
            nc.sync.dma_start(o_dram[:], s_o[:])

    nc.compile()
    return nc


def _get_nc(qrows=QROWS, dbn=DBN):
    key = (qrows, dbn)
    if key not in _NC_CACHE:
        _NC_CACHE[key] = _build_nc(qrows, dbn)
    return _NC_CACHE[key]


def _split3(a):
    """fp32 array -> (hi, mid, lo) bf16 triple, hi+mid+lo ~ a to ~2^-27 |a|."""
    hi = a.astype(ml_dtypes.bfloat16)
    r = a - hi.astype(np.float32)
    mid = r.astype(ml_dtypes.bfloat16)
    lo = (r - mid.astype(np.float32)).astype(ml_dtypes.bfloat16)
    return hi, mid, lo


def _build_lhs(q):
    """q [Q, 3] fp32 -> stationary operand [24, Q] bf16."""
    qq = np.ascontiguousarray(q.T)  # [3, Q]
    A, AL, AL2 = _split3(qq)
    ones = np.ones((3, q.shape[0]), dtype=ml_dtypes.bfloat16)
    q2 = (q.astype(np.float64) ** 2).sum(axis=1).astype(np.float32)[None, :]
    q2h, q2l, q2l2 = _split3(q2)
    return np.concatenate([A, A, A, AL, AL, AL2, ones, q2h, q2l, q2l2], axis=0)


def _build_rhs(d):
    """d [Dn, 3] fp32 -> moving operand [24, Dn] bf16."""
    t = np.ascontiguousarray(d.T) * np.float32(-2.0)  # [3, Dn]
    C, E, E2 = _split3(t)
    d2 = (d.astype(np.float64) ** 2).sum(axis=1).astype(np.float32)[None, :]
    d2h, d2l, d2l2 = _split3(d2)
    ones = np.ones((3, d.shape[0]), dtype=ml_dtypes.bfloat16)
    return np.concatenate([C, E, E2, C, E, C, d2h, d2l, d2l2, ones], axis=0)


def _unpack_mins(o):
    """o [128, 2*nblk] fp32 per-block dist-min pairs -> [nblk*128] row mins."""
    v = np.asarray(o).reshape(BLKP, -1, 2).min(axis=2)  # [p, blk]
    return v.T.reshape(-1)  # row = blk*128 + p


def kernel(x, y):
    from concourse.bass_utils import run_bass_kernel_spmd

    x = np.asarray(x, dtype=np.float32)
    y = np.asarray(y, dtype=np.float32)
    assert x.shape == (B, N, D) and y.shape == (B, M, D)

    in_maps = []
    rhs_y = [_build_rhs(y[b]) for b in range(B)]
    rhs_x = [_build_rhs(x[b]) for b in range(B)]
    for c in range(NCORES):
        b, h = divmod(c, 2)
        sl = slice(h * QROWS, (h + 1) * QROWS)
        in_maps.append(
            {
                "lx": _build_lhs(x[b, sl]),
                "ry": rhs_y[b],
                "ly": _build_lhs(y[b, sl]),
                "rx": rhs_x[b],
            }
        )

    nc = _get_nc()
    res = run_bass_kernel_spmd(nc, in_maps, core_ids=list(range(NCORES)))

    total = 0.0
    for b in range(B):
        minx = np.empty(N, dtype=np.float64)
        miny = np.empty(M, dtype=np.float64)
        for h in range(2):
            r = res.results[2 * b + h]
            sl = slice(h * QROWS, (h + 1) * QROWS)
            minx[sl] = _unpack_mins(r["ox"])
            miny[sl] = _unpack_mins(r["oy"])
        total += minx.mean() + miny.mean()

    return np.float32(total / B)



# revision 7
# speedup vs baseline: 3.6970x; 3.6970x over previous
"""Chamfer loss kernel for Trainium2 (8 NeuronCores, Bass/Tile).

Problem: x (4, 8192, 3), y (4, 8192, 3) fp32.
  dist[b,i,j] = ||x_bi||^2 + ||y_bj||^2 - 2 x_bi . y_bj
  out = mean_b( mean_i min_j dist + mean_j min_i dist )

Sharding: 8 cores = 4 batches x 2 halves. Core (b, h) computes
  - x->y mins for x rows [h*4096, (h+1)*4096) of batch b vs ALL y[b]
  - y->x mins for y rows [h*4096, (h+1)*4096) of batch b vs ALL x[b]
so each core owns full rows of output; no cross-core reduction needed.

Transfer-minimal formulation (the dispatch wall is dominated by the axon
tunnel: ~90 ms latency floor + ~50 MB/s, so bytes moved matter far more
than device cycles):
  - The host uploads ONE 13-row bf16 "piece" per tensor half per core:
    rows = [A, AL, AL2, n2h, n2l, n2l2, ones] where A+AL+AL2 ~ coords.T
    (3-way bf16 split, accurate to ~2^-27) and n2* is the 3-way split of
    -||p||^2/2 (computed in f64 on host). 104 KiB/core/tensor -> 1.6 MiB
    total vs 9.4 MiB for pre-built 24-row operands. (The ones row rides
    along because compute-engine memsets at partition offset 21 fail BIR
    verification; DMA row copies have no partition-alignment rule.)
  - Matmul computes H = x.y - (||x||^2+||y||^2)/2 = -dist/2. Folding the
    -1/2 into the norm rows on the host makes EVERY operand row a pure
    byte copy of piece rows, so operand assembly is DMA-only (no
    ACT/DVE work): lhs rows [A,A,A,AL,AL,AL2,n2,ones], rhs rows
    [A,AL,AL2,A,AL,A,ones,n2] pair up to give the 6 retained cross
    products + both norms. min_j dist = -2 max_j H.
  - Each core uploads only its OWN halves; full-batch operands are
    reconstructed on device via a pair AllGather (cores {2b, 2b+1}) of
    the raw pieces over NeuronLink. db column order after the gather is
    irrelevant: max over db points is order-agnostic.
  - The drain uses max instead of min (H values cluster just below 0 for
    near neighbors, so the fp16 PSUM->SBUF rounding stays harmless, same
    argument as the min formulation). Per-row maxes are folded and
    row-summed ON DEVICE, so each core fetches back only [128, 2] f32
    (8 KiB total vs 512 KiB).

Drain pipeline per 128-row block (PSUM in [128, 2048] 4-bank groups):
even blocks ACT-copy all 4 groups to fp16 in SBUF and DVE tree-maxes
them; odd blocks DVE-direct-reduce group 0 from PSUM and ACT-copy the
remaining 3 (balances ACT vs DVE element traffic).
"""

import numpy as np
import ml_dtypes

B = 4
N = 8192  # x points per batch
M = 8192  # y points per batch
D = 3
NCORES = 8

QROWS = 4096  # query rows per core (half of a batch's points)
DBN = 8192  # database points scanned per query
PROWS = 13  # uploaded piece rows: A(3), AL(3), AL2(3), n2h, n2l, n2l2, ones
KDIM = 24  # augmented contraction dim
BLKP = 128  # query rows per matmul block (PSUM partitions)
FREE = 512  # matmul free size (one PSUM fp32 bank)
G2 = 2048  # PSUM drain group (4 banks)
NBLK = QROWS // BLKP  # 32

_NC_CACHE = {}
_RUNNER_CACHE = {}


def _build_nc(repeat=1):
    from contextlib import ExitStack

    import concourse.tile as tile
    from concourse import bacc, mybir

    bf16 = mybir.dt.bfloat16
    f16 = mybir.dt.float16
    f32 = mybir.dt.float32
    mx = mybir.AluOpType.max

    nc = bacc.Bacc(
        "TRN2", target_bir_lowering=False, debug=False, num_devices=NCORES
    )
    px = nc.dram_tensor("px", [PROWS, QROWS], bf16, kind="ExternalInput")
    py = nc.dram_tensor("py", [PROWS, QROWS], bf16, kind="ExternalInput")
    o = nc.dram_tensor("o", [BLKP, 2], f32, kind="ExternalOutput")

    NEG = -float(np.finfo(np.float32).max)

    with tile.TileContext(nc) as tc, ExitStack() as ctx:
        dram = ctx.enter_context(tc.tile_pool(name="dram", bufs=1, space="DRAM"))
        cpool = ctx.enter_context(tc.tile_pool(name="consts", bufs=1))
        ppool = ctx.enter_context(tc.tile_pool(name="psum", bufs=2, space="PSUM"))
        spool = ctx.enter_context(tc.tile_pool(name="scratch", bufs=3))
        opool = ctx.enter_context(tc.tile_pool(name="outs", bufs=1))

        # -- exchange raw pieces within each batch pair over NeuronLink.
        # Collectives need DRAM bounce buffers (not I/O tensors directly).
        bx = dram.tile([PROWS, QROWS], bf16, tag="bx")
        by = dram.tile([PROWS, QROWS], bf16, tag="by")
        gx = dram.tile([2 * PROWS, QROWS], bf16, tag="gx")
        gy = dram.tile([2 * PROWS, QROWS], bf16, tag="gy")
        groups = [[0, 1], [2, 3], [4, 5], [6, 7]]
        nc.gpsimd.dma_start(bx[:], px[:])
        nc.gpsimd.dma_start(by[:], py[:])
        nc.gpsimd.collective_compute(
            "AllGather",
            mybir.AluOpType.bypass,
            replica_groups=groups,
            ins=[bx.opt()],
            outs=[gx.opt()],
        )
        nc.gpsimd.collective_compute(
            "AllGather",
            mybir.AluOpType.bypass,
            replica_groups=groups,
            ins=[by.opt()],
            outs=[gy.opt()],
        )

        # -- operand assembly: pure DMA row copies + ones memsets.
        # lhs rows [A,A,A, AL,AL, AL2, n2(3), ones(3)] from own piece;
        # rhs rows [A,AL,AL2, A,AL, A, ones(3), n2(3)] per gathered half.
        # Row-k products: A.A + A.AL' + A.AL2' + AL.A' + AL.AL' + AL2.A'
        # + n2_q.1 + 1.n2_d = x.y - (|x|^2+|y|^2)/2 = H = -dist/2.
        lhs_x = cpool.tile([KDIM, QROWS], bf16, tag="lhs_x")
        lhs_y = cpool.tile([KDIM, QROWS], bf16, tag="lhs_y")
        rhs_x = cpool.tile([KDIM, DBN], bf16, tag="rhs_x")
        rhs_y = cpool.tile([KDIM, DBN], bf16, tag="rhs_y")

        for lhs, piece in ((lhs_x, px), (lhs_y, py)):
            nc.sync.dma_start(lhs[0:3, :], piece[0:3, :])
            nc.sync.dma_start(lhs[3:6, :], piece[0:3, :])
            nc.sync.dma_start(lhs[6:9, :], piece[0:3, :])
            nc.sync.dma_start(lhs[9:12, :], piece[3:6, :])
            nc.sync.dma_start(lhs[12:15, :], piece[3:6, :])
            nc.sync.dma_start(lhs[15:18, :], piece[6:9, :])
            nc.sync.dma_start(lhs[18:21, :], piece[9:12, :])
            for r in range(3):
                nc.sync.dma_start(lhs[21 + r : 22 + r, :], piece[12:13, :])
        for rhs, g in ((rhs_x, gx), (rhs_y, gy)):
            for hb in range(2):
                r0 = hb * PROWS
                cs = slice(hb * QROWS, (hb + 1) * QROWS)
                nc.sync.dma_start(rhs[0:9, cs], g[r0 : r0 + 9, :])
                nc.sync.dma_start(rhs[9:15, cs], g[r0 : r0 + 6, :])
                nc.sync.dma_start(rhs[15:18, cs], g[r0 : r0 + 3, :])
                nc.sync.dma_start(rhs[21:24, cs], g[r0 + 9 : r0 + 12, :])
                for r in range(3):
                    nc.sync.dma_start(
                        rhs[18 + r : 19 + r, cs], g[r0 + 12 : r0 + 13, :]
                    )

        s_out = opool.tile([BLKP, 2], f32, tag="out")

        loop_ctx = tc.For_i(0, repeat, 1) if repeat > 1 else None
        if loop_ctx is not None:
            ctx.enter_context(loop_ctx)

        for col, (lhs, rhs) in enumerate(((lhs_x, rhs_y), (lhs_y, rhs_x))):
            # s_o cols [0:NBLK] = tree maxes, [NBLK:2*NBLK] = direct maxes
            s_o = opool.tile([BLKP, 2 * NBLK], f32, tag=f"so{col}")
            nc.gpsimd.memset(s_o[:], NEG)
            for blk in range(NBLK):
                lhs_blk = lhs[:, blk * BLKP : (blk + 1) * BLKP]

                def fill2(grp):
                    ps = ppool.tile([BLKP, G2], f32, tag="ps2")
                    for t in range(G2 // FREE):
                        c0 = grp * G2 + t * FREE
                        nc.tensor.matmul(
                            ps[:, t * FREE : (t + 1) * FREE],
                            lhs_blk,
                            rhs[:, c0 : c0 + FREE],
                            start=True,
                            stop=True,
                        )
                    return ps

                ngroups = DBN // G2  # 4
                direct = blk % 2 == 1
                g0 = 0
                if direct:
                    ps = fill2(0)
                    nc.vector.tensor_reduce(
                        s_o[:, NBLK + blk : NBLK + blk + 1],
                        ps[:],
                        axis=mybir.AxisListType.X,
                        op=mx,
                    )
                    g0 = 1
                na = ngroups - g0
                S = spool.tile([BLKP, na * G2], f16, tag=f"s16_{na}")
                for grp in range(g0, ngroups):
                    ps = fill2(grp)
                    o0 = (grp - g0) * G2
                    nc.scalar.copy(S[:, o0 : o0 + G2], ps[:])
                if na == 3:
                    # 6144 wide: fold the odd group in with two TTs
                    T1 = spool.tile([BLKP, G2], f16, tag="t6a")
                    nc.vector.tensor_tensor(
                        T1[:], S[:, 0:G2], S[:, G2 : 2 * G2], op=mx
                    )
                    T2 = spool.tile([BLKP, G2], f16, tag="t6b")
                    nc.vector.tensor_tensor(
                        T2[:], T1[:], S[:, 2 * G2 : 3 * G2], op=mx
                    )
                    cur, w = T2, G2
                else:
                    cur, w = S, na * G2
                while w > 1024:
                    nxt = spool.tile([BLKP, w // 2], f16, tag=f"t{w // 2}")
                    nc.vector.tensor_tensor(
                        nxt[:], cur[:, 0 : w // 2], cur[:, w // 2 : w], op=mx
                    )
                    cur, w = nxt, w // 2
                nc.vector.tensor_reduce(
                    s_o[:, blk : blk + 1],
                    cur[:],
                    axis=mybir.AxisListType.X,
                    op=mx,
                )
            # per-row max over (tree, direct) halves, then sum rows' maxes
            fold = spool.tile([BLKP, NBLK], f32, tag=f"fold{col}")
            nc.vector.tensor_tensor(
                fold[:], s_o[:, 0:NBLK], s_o[:, NBLK : 2 * NBLK], op=mx
            )
            nc.vector.tensor_reduce(
                s_out[:, col : col + 1],
                fold[:],
                axis=mybir.AxisListType.X,
                op=mybir.AluOpType.add,
            )
        nc.sync.dma_start(o[:], s_out[:])

    nc.compile()
    return nc


def _get_nc():
    if "nc" not in _NC_CACHE:
        _NC_CACHE["nc"] = _build_nc()
    return _NC_CACHE["nc"]


def _split3(a):
    """fp32 array -> (hi, mid, lo) bf16 triple, hi+mid+lo ~ a to ~2^-27 |a|."""
    hi = a.astype(ml_dtypes.bfloat16)
    r = a - hi.astype(np.float32)
    mid = r.astype(ml_dtypes.bfloat16)
    lo = (r - mid.astype(np.float32)).astype(ml_dtypes.bfloat16)
    return hi, mid, lo


def _piece(p):
    """p [Q, 3] fp32 -> uploaded piece [13, Q] bf16."""
    P = np.ascontiguousarray(p.T)  # [3, Q]
    A, AL, AL2 = _split3(P)
    h2 = (-0.5 * (p.astype(np.float64) ** 2).sum(axis=1)).astype(np.float32)
    n2h, n2l, n2l2 = _split3(h2[None, :])
    ones = np.ones((1, p.shape[0]), dtype=ml_dtypes.bfloat16)
    return np.concatenate([A, AL, AL2, n2h, n2l, n2l2, ones], axis=0)


def _make_in_maps(x, y):
    in_maps = []
    for c in range(NCORES):
        b, h = divmod(c, 2)
        sl = slice(h * QROWS, (h + 1) * QROWS)
        in_maps.append({"px": _piece(x[b, sl]), "py": _piece(y[b, sl])})
    return in_maps


def _get_runner(nc):
    """Build (once) a cached jitted SPMD dispatcher for `nc`.

    Same lowering as concourse.bass_utils.run_bass_kernel_spmd under axon
    (shard_map over 8 cores of a bass_exec custom call), but the jitted
    callable is reused across kernel() invocations, saving the per-call
    retrace/relower (~100 ms).
    """
    key = id(nc)
    if key in _RUNNER_CACHE:
        return _RUNNER_CACHE[key]

    import jax
    import numpy as np
    from jax.sharding import Mesh, PartitionSpec

    try:
        from jax.experimental.shard_map import shard_map
    except ImportError:  # newer jax
        from jax.shard_map import shard_map  # type: ignore

    from concourse import mybir
    from concourse.bass2jax import (
        _bass_exec_p,
        install_neuronx_cc_hook,
        partition_id_tensor,
    )

    install_neuronx_cc_hook()

    partition_name = (
        nc.partition_id_tensor.name if nc.partition_id_tensor else None
    )
    in_names = []
    out_names = []
    out_avals = []
    zero_outs = []
    for alloc in nc.m.functions[0].allocations:
        if not isinstance(alloc, mybir.MemoryLocationSet):
            continue
        name = alloc.memorylocations[0].name
        if alloc.kind == "ExternalInput":
            if name != partition_name:
                in_names.append(name)
        elif alloc.kind == "ExternalOutput":
            shape = tuple(alloc.tensor_shape)
            dtype = mybir.dt.np(alloc.dtype)
            out_names.append(name)
            out_avals.append(jax.core.ShapedArray(shape, dtype))
            zero_outs.append(np.zeros(shape, dtype))
    n_params = len(in_names)
    n_outs = len(out_avals)
    all_in_names = list(in_names) + list(out_names)
    if partition_name is not None:
        all_in_names.append(partition_name)
    donate = tuple(range(n_params, n_params + n_outs))

    def _body(*args):
        operands = list(args)
        if partition_name is not None:
            operands.append(partition_id_tensor())
        outs = _bass_exec_p.bind(
            *operands,
            out_avals=tuple(out_avals),
            in_names=tuple(all_in_names),
            out_names=tuple(out_names),
            lowering_input_output_aliases=(),
            sim_require_finite=True,
            sim_require_nnan=True,
            nc=nc,
        )
        return tuple(outs)

    devices = jax.devices()[:NCORES]
    mesh = Mesh(np.asarray(devices), ("core",))
    in_specs = (PartitionSpec("core"),) * (n_params + n_outs)
    out_specs = (PartitionSpec("core"),) * n_outs
    sharded = jax.jit(
        shard_map(
            _body, mesh=mesh, in_specs=in_specs, out_specs=out_specs,
            check_rep=False,
        ),
        donate_argnums=donate,
        keep_unused=True,
    )

    def run(in_maps):
        concat_in = [
            np.concatenate([m[name] for m in in_maps], axis=0)
            for name in in_names
        ]
        concat_zeros = [
            np.zeros((NCORES * z.shape[0], *z.shape[1:]), z.dtype)
            for z in zero_outs
        ]
        out_arrs = sharded(*concat_in, *concat_zeros)
        return [
            {
                name: np.asarray(out_arrs[i]).reshape(
                    NCORES, *out_avals[i].shape
                )[c]
                for i, name in enumerate(out_names)
            }
            for c in range(NCORES)
        ]

    _RUNNER_CACHE[key] = run
    return run


def _finish(results):
    """Per-core [128, 2] f32 row-sums of max_j H -> scalar chamfer loss."""
    total = 0.0
    for c in range(NCORES):
        total += np.asarray(results[c]["o"], dtype=np.float64).sum()
    return np.float32(-2.0 * total / (N * B))


def kernel(x, y):
    x = np.asarray(x, dtype=np.float32)
    y = np.asarray(y, dtype=np.float32)
    assert x.shape == (B, N, D) and y.shape == (B, M, D)

    in_maps = _make_in_maps(x, y)
    nc = _get_nc()
    run = _get_runner(nc)
    return _finish(run(in_maps))


# revision 12
# speedup vs baseline: 7.2033x; 1.9484x over previous
"""Chamfer loss kernel for Trainium2 (8 NeuronCores, Bass/Tile).

Problem: x (4, 8192, 3), y (4, 8192, 3) fp32.
  dist[b,i,j] = ||x_bi||^2 + ||y_bj||^2 - 2 x_bi . y_bj
  out = mean_b( mean_i min_j dist + mean_j min_i dist )

Sharding: 8 cores = 4 batches x 2 halves. Core (b, h) computes
  - x->y mins for x rows [h*4096, (h+1)*4096) of batch b vs ALL y[b]
  - y->x mins for y rows [h*4096, (h+1)*4096) of batch b vs ALL x[b]
so each core owns full rows of output; no cross-core reduction needed.

Transfer-minimal formulation (the dispatch wall is dominated by the axon
tunnel: ~90 ms latency floor + ~50 MB/s, so bytes moved matter far more
than device cycles):
  - The host uploads ONE 13-row bf16 "piece" per tensor half per core:
    rows = [A, AL, AL2, n2h, n2l, n2l2, ones] where A+AL+AL2 ~ coords.T
    (3-way bf16 split, accurate to ~2^-27) and n2* is the 3-way split of
    -||p||^2/2 (computed in f64 on host). 104 KiB/core/tensor -> 1.6 MiB
    total vs 9.4 MiB for pre-built 24-row operands. (The ones row rides
    along because compute-engine memsets at partition offset 21 fail BIR
    verification; DMA row copies have no partition-alignment rule.)
  - Matmul computes H = x.y - (||x||^2+||y||^2)/2 = -dist/2. Folding the
    -1/2 into the norm rows on the host makes EVERY operand row a pure
    byte copy of piece rows, so operand assembly is DMA-only (no
    ACT/DVE work): lhs rows [A,A,A,AL,AL,AL2,n2,ones], rhs rows
    [A,AL,AL2,A,AL,A,ones,n2] pair up to give the 6 retained cross
    products + both norms. min_j dist = -2 max_j H.
  - Each core uploads only its OWN halves; full-batch operands are
    reconstructed on device via a pair AllGather (cores {2b, 2b+1}) of
    the raw pieces over NeuronLink. db column order after the gather is
    irrelevant: max over db points is order-agnostic.
  - The drain uses max instead of min (H values cluster just below 0 for
    near neighbors, so the fp16 PSUM->SBUF rounding stays harmless, same
    argument as the min formulation). Per-row maxes are folded and
    row-summed ON DEVICE, so each core fetches back only [128, 2] f32
    (8 KiB total vs 512 KiB).

Drain pipeline per 128-row block (PSUM in [128, 2048] 4-bank groups):
even blocks ACT-copy all 4 groups to fp16 in SBUF and DVE tree-maxes
them; odd blocks DVE-direct-reduce group 0 from PSUM and ACT-copy the
remaining 3 (balances ACT vs DVE element traffic).
"""

import numpy as np
import ml_dtypes

B = 4
N = 8192  # x points per batch
M = 8192  # y points per batch
D = 3
NCORES = 8

QROWS = 4096  # query rows per core (half of a batch's points)
DBN = 8192  # database points scanned per query
PROWS = 13  # bf16w3 piece rows: A(3), AL(3), AL2(3), n2h, n2l, n2l2, ones
PROWS_F16 = 9  # f16w2 piece rows: A(3), AL(3), n2h, n2l, ones
KDIM = 24  # augmented contraction dim (bf16w3; f16w2 uses 13)
BLKP = 128  # query rows per matmul block (PSUM partitions)
FREE = 512  # matmul free size (one PSUM fp32 bank)
G2 = 2048  # PSUM drain group (4 banks)
NBLK = QROWS // BLKP  # 32

_NC_CACHE = {}
_RUNNER_CACHE = {}

# "bf16w3": 13-row bf16 pieces (3-way splits, K=24), separate px/py inputs.
# "f16w2": 9-row f16 pieces (2-way splits, K=13), one merged pxy input +
#          single AllGather; ~30% less upload, ~10x coarser (still ~200x
#          inside the 2e-2 gate) numerics.
VARIANT = "f16w2"


def _build_nc(repeat=1, variant=None):
    from contextlib import ExitStack

    import concourse.tile as tile
    from concourse import bacc, mybir

    variant = VARIANT if variant is None else variant
    bf16 = mybir.dt.bfloat16
    f16 = mybir.dt.float16
    f32 = mybir.dt.float32
    mx = mybir.AluOpType.max
    groups = [[0, 1], [2, 3], [4, 5], [6, 7]]

    nc = bacc.Bacc(
        "TRN2", target_bir_lowering=False, debug=False, num_devices=NCORES
    )
    o = nc.dram_tensor("o", [BLKP, 2], f32, kind="ExternalOutput")

    NEG = -float(np.finfo(np.float32).max)

    with tile.TileContext(nc) as tc, ExitStack() as ctx:
        dram = ctx.enter_context(tc.tile_pool(name="dram", bufs=1, space="DRAM"))
        cpool = ctx.enter_context(tc.tile_pool(name="consts", bufs=1))
        ppool = ctx.enter_context(tc.tile_pool(name="psum", bufs=2, space="PSUM"))
        spool = ctx.enter_context(tc.tile_pool(name="scratch", bufs=3))
        opool = ctx.enter_context(tc.tile_pool(name="outs", bufs=1))

        if variant == "bf16w3":
            kdim = 24
            px = nc.dram_tensor("px", [PROWS, QROWS], bf16, kind="ExternalInput")
            py = nc.dram_tensor("py", [PROWS, QROWS], bf16, kind="ExternalInput")

            # -- exchange raw pieces within each batch pair over NeuronLink.
            # Collectives need DRAM bounce buffers (not I/O tensors directly).
            bx = dram.tile([PROWS, QROWS], bf16, tag="bx")
            by = dram.tile([PROWS, QROWS], bf16, tag="by")
            gx = dram.tile([2 * PROWS, QROWS], bf16, tag="gx")
            gy = dram.tile([2 * PROWS, QROWS], bf16, tag="gy")
            nc.gpsimd.dma_start(bx[:], px[:])
            nc.gpsimd.dma_start(by[:], py[:])
            nc.gpsimd.collective_compute(
                "AllGather",
                mybir.AluOpType.bypass,
                replica_groups=groups,
                ins=[bx.opt()],
                outs=[gx.opt()],
            )
            nc.gpsimd.collective_compute(
                "AllGather",
                mybir.AluOpType.bypass,
                replica_groups=groups,
                ins=[by.opt()],
                outs=[gy.opt()],
            )

            # -- operand assembly: pure DMA row copies.
            # lhs rows [A,A,A, AL,AL, AL2, n2(3), ones(3)] from own piece;
            # rhs rows [A,AL,AL2, A,AL, A, ones(3), n2(3)] per gathered half.
            # Row-k products: A.A + A.AL' + A.AL2' + AL.A' + AL.AL' + AL2.A'
            # + n2_q.1 + 1.n2_d = x.y - (|x|^2+|y|^2)/2 = H = -dist/2.
            lhs_x = cpool.tile([kdim, QROWS], bf16, tag="lhs_x")
            lhs_y = cpool.tile([kdim, QROWS], bf16, tag="lhs_y")
            rhs_x = cpool.tile([kdim, DBN], bf16, tag="rhs_x")
            rhs_y = cpool.tile([kdim, DBN], bf16, tag="rhs_y")

            for lhs, piece in ((lhs_x, px), (lhs_y, py)):
                nc.sync.dma_start(lhs[0:3, :], piece[0:3, :])
                nc.sync.dma_start(lhs[3:6, :], piece[0:3, :])
                nc.sync.dma_start(lhs[6:9, :], piece[0:3, :])
                nc.sync.dma_start(lhs[9:12, :], piece[3:6, :])
                nc.sync.dma_start(lhs[12:15, :], piece[3:6, :])
                nc.sync.dma_start(lhs[15:18, :], piece[6:9, :])
                nc.sync.dma_start(lhs[18:21, :], piece[9:12, :])
                for r in range(3):
                    nc.sync.dma_start(lhs[21 + r : 22 + r, :], piece[12:13, :])
            for rhs, g in ((rhs_x, gx), (rhs_y, gy)):
                for hb in range(2):
                    r0 = hb * PROWS
                    cs = slice(hb * QROWS, (hb + 1) * QROWS)
                    nc.sync.dma_start(rhs[0:9, cs], g[r0 : r0 + 9, :])
                    nc.sync.dma_start(rhs[9:15, cs], g[r0 : r0 + 6, :])
                    nc.sync.dma_start(rhs[15:18, cs], g[r0 : r0 + 3, :])
                    nc.sync.dma_start(rhs[21:24, cs], g[r0 + 9 : r0 + 12, :])
                    for r in range(3):
                        nc.sync.dma_start(
                            rhs[18 + r : 19 + r, cs], g[r0 + 12 : r0 + 13, :]
                        )
        else:  # f16w2
            kdim = 13
            pr = PROWS_F16  # 9: A(3), AL(3), n2h, n2l, one
            pxy = nc.dram_tensor(
                "pxy", [2 * pr, QROWS], f16, kind="ExternalInput"
            )

            bxy = dram.tile([2 * pr, QROWS], f16, tag="bxy")
            gxy = dram.tile([4 * pr, QROWS], f16, tag="gxy")
            nc.gpsimd.dma_start(bxy[:], pxy[:])
            nc.gpsimd.collective_compute(
                "AllGather",
                mybir.AluOpType.bypass,
                replica_groups=groups,
                ins=[bxy.opt()],
                outs=[gxy.opt()],
            )

            # lhs rows [A,A,AL, n2h, n2l, one, one] from own piece;
            # rhs rows [A,AL,A, one, one, n2h, n2l] per gathered half.
            # Row-k products: A.A' + A.AL' + AL.A' + n2_q.1 + 1.n2_d = H.
            lhs_x = cpool.tile([kdim, QROWS], f16, tag="lhs_x")
            lhs_y = cpool.tile([kdim, QROWS], f16, tag="lhs_y")
            rhs_x = cpool.tile([kdim, DBN], f16, tag="rhs_x")
            rhs_y = cpool.tile([kdim, DBN], f16, tag="rhs_y")

            for lhs, r0 in ((lhs_x, 0), (lhs_y, pr)):
                nc.sync.dma_start(lhs[0:3, :], pxy[r0 : r0 + 3, :])
                nc.sync.dma_start(lhs[3:6, :], pxy[r0 : r0 + 3, :])
                nc.sync.dma_start(lhs[6:9, :], pxy[r0 + 3 : r0 + 6, :])
                nc.sync.dma_start(lhs[9:11, :], pxy[r0 + 6 : r0 + 8, :])
                nc.sync.dma_start(lhs[11:12, :], pxy[r0 + 8 : r0 + 9, :])
                nc.sync.dma_start(lhs[12:13, :], pxy[r0 + 8 : r0 + 9, :])
            for rhs, po in ((rhs_x, 0), (rhs_y, pr)):
                for hb in range(2):
                    r0 = hb * 2 * pr + po
                    cs = slice(hb * QROWS, (hb + 1) * QROWS)
                    nc.sync.dma_start(rhs[0:6, cs], gxy[r0 : r0 + 6, :])
                    nc.sync.dma_start(rhs[6:9, cs], gxy[r0 : r0 + 3, :])
                    nc.sync.dma_start(rhs[9:10, cs], gxy[r0 + 8 : r0 + 9, :])
                    nc.sync.dma_start(rhs[10:11, cs], gxy[r0 + 8 : r0 + 9, :])
                    nc.sync.dma_start(rhs[11:13, cs], gxy[r0 + 6 : r0 + 8, :])

        s_out = opool.tile([BLKP, 2], f32, tag="out")

        loop_ctx = tc.For_i(0, repeat, 1) if repeat > 1 else None
        if loop_ctx is not None:
            ctx.enter_context(loop_ctx)

        for col, (lhs, rhs) in enumerate(((lhs_x, rhs_y), (lhs_y, rhs_x))):
            # s_o cols [0:NBLK] = tree maxes, [NBLK:2*NBLK] = direct maxes
            s_o = opool.tile([BLKP, 2 * NBLK], f32, tag=f"so{col}")
            nc.gpsimd.memset(s_o[:], NEG)
            for blk in range(NBLK):
                lhs_blk = lhs[:, blk * BLKP : (blk + 1) * BLKP]

                def fill2(grp):
                    ps = ppool.tile([BLKP, G2], f32, tag="ps2")
                    for t in range(G2 // FREE):
                        c0 = grp * G2 + t * FREE
                        nc.tensor.matmul(
                            ps[:, t * FREE : (t + 1) * FREE],
                            lhs_blk,
                            rhs[:, c0 : c0 + FREE],
                            start=True,
                            stop=True,
                        )
                    return ps

                ngroups = DBN // G2  # 4
                direct = blk % 2 == 1
                g0 = 0
                if direct:
                    ps = fill2(0)
                    nc.vector.tensor_reduce(
                        s_o[:, NBLK + blk : NBLK + blk + 1],
                        ps[:],
                        axis=mybir.AxisListType.X,
                        op=mx,
                    )
                    g0 = 1
                na = ngroups - g0
                S = spool.tile([BLKP, na * G2], f16, tag=f"s16_{na}")
                for grp in range(g0, ngroups):
                    ps = fill2(grp)
                    o0 = (grp - g0) * G2
                    nc.scalar.copy(S[:, o0 : o0 + G2], ps[:])
                if na == 3:
                    # 6144 wide: fold the odd group in with two TTs
                    T1 = spool.tile([BLKP, G2], f16, tag="t6a")
                    nc.vector.tensor_tensor(
                        T1[:], S[:, 0:G2], S[:, G2 : 2 * G2], op=mx
                    )
                    T2 = spool.tile([BLKP, G2], f16, tag="t6b")
                    nc.vector.tensor_tensor(
                        T2[:], T1[:], S[:, 2 * G2 : 3 * G2], op=mx
                    )
                    cur, w = T2, G2
                else:
                    cur, w = S, na * G2
                while w > 1024:
                    nxt = spool.tile([BLKP, w // 2], f16, tag=f"t{w // 2}")
                    nc.vector.tensor_tensor(
                        nxt[:], cur[:, 0 : w // 2], cur[:, w // 2 : w], op=mx
                    )
                    cur, w = nxt, w // 2
                nc.vector.tensor_reduce(
                    s_o[:, blk : blk + 1],
                    cur[:],
                    axis=mybir.AxisListType.X,
                    op=mx,
                )
            # per-row max over (tree, direct) halves, then sum rows' maxes
            fold = spool.tile([BLKP, NBLK], f32, tag=f"fold{col}")
            nc.vector.tensor_tensor(
                fold[:], s_o[:, 0:NBLK], s_o[:, NBLK : 2 * NBLK], op=mx
            )
            nc.vector.tensor_reduce(
                s_out[:, col : col + 1],
                fold[:],
                axis=mybir.AxisListType.X,
                op=mybir.AluOpType.add,
            )
        nc.sync.dma_start(o[:], s_out[:])

    nc.compile()
    return nc


def _get_nc():
    if VARIANT not in _NC_CACHE:
        _NC_CACHE[VARIANT] = _build_nc()
    return _NC_CACHE[VARIANT]


def _split3(a):
    """fp32 array -> (hi, mid, lo) bf16 triple, hi+mid+lo ~ a to ~2^-27 |a|."""
    hi = a.astype(ml_dtypes.bfloat16)
    r = a - hi.astype(np.float32)
    mid = r.astype(ml_dtypes.bfloat16)
    lo = (r - mid.astype(np.float32)).astype(ml_dtypes.bfloat16)
    return hi, mid, lo


def _split2_f16(a):
    """fp32 array -> (hi, lo) f16 pair, hi+lo ~ a to ~2^-23 |a|."""
    hi = a.astype(np.float16)
    lo = (a - hi.astype(np.float32)).astype(np.float16)
    return hi, lo


def _piece(p):
    """p [Q, 3] fp32 -> uploaded piece [13, Q] bf16."""
    P = np.ascontiguousarray(p.T)  # [3, Q]
    A, AL, AL2 = _split3(P)
    h2 = (-0.5 * (p.astype(np.float64) ** 2).sum(axis=1)).astype(np.float32)
    n2h, n2l, n2l2 = _split3(h2[None, :])
    ones = np.ones((1, p.shape[0]), dtype=ml_dtypes.bfloat16)
    return np.concatenate([A, AL, AL2, n2h, n2l, n2l2, ones], axis=0)


def _piece_f16(p):
    """p [Q, 3] fp32 -> uploaded piece [9, Q] f16."""
    P = np.ascontiguousarray(p.T)  # [3, Q]
    A, AL = _split2_f16(P)
    h2 = (-0.5 * (p.astype(np.float64) ** 2).sum(axis=1)).astype(np.float32)
    n2h, n2l = _split2_f16(h2[None, :])
    ones = np.ones((1, p.shape[0]), dtype=np.float16)
    return np.concatenate([A, AL, n2h, n2l, ones], axis=0)


def _make_in_maps(x, y):
    in_maps = []
    for c in range(NCORES):
        b, h = divmod(c, 2)
        sl = slice(h * QROWS, (h + 1) * QROWS)
        if VARIANT == "bf16w3":
            in_maps.append({"px": _piece(x[b, sl]), "py": _piece(y[b, sl])})
        else:
            in_maps.append(
                {
                    "pxy": np.concatenate(
                        [_piece_f16(x[b, sl]), _piece_f16(y[b, sl])], axis=0
                    )
                }
            )
    return in_maps


def _get_runner(nc):
    """Build (once) a cached jitted SPMD dispatcher for `nc`.

    Same lowering as concourse.bass_utils.run_bass_kernel_spmd under axon
    (shard_map over 8 cores of a bass_exec custom call), but the jitted
    callable is reused across kernel() invocations, saving the per-call
    retrace/relower (~100 ms).
    """
    key = id(nc)
    if key in _RUNNER_CACHE:
        return _RUNNER_CACHE[key]

    import jax
    import numpy as np
    from jax.sharding import Mesh, PartitionSpec

    try:
        from jax.experimental.shard_map import shard_map
    except ImportError:  # newer jax
        from jax.shard_map import shard_map  # type: ignore

    from concourse import mybir
    from concourse.bass2jax import (
        _bass_exec_p,
        install_neuronx_cc_hook,
        partition_id_tensor,
    )

    install_neuronx_cc_hook()

    partition_name = (
        nc.partition_id_tensor.name if nc.partition_id_tensor else None
    )
    in_names = []
    out_names = []
    out_avals = []
    zero_outs = []
    for alloc in nc.m.functions[0].allocations:
        if not isinstance(alloc, mybir.MemoryLocationSet):
            continue
        name = alloc.memorylocations[0].name
        if alloc.kind == "ExternalInput":
            if name != partition_name:
                in_names.append(name)
        elif alloc.kind == "ExternalOutput":
            shape = tuple(alloc.tensor_shape)
            dtype = mybir.dt.np(alloc.dtype)
            out_names.append(name)
            out_avals.append(jax.core.ShapedArray(shape, dtype))
            zero_outs.append(np.zeros(shape, dtype))
    n_params = len(in_names)
    n_outs = len(out_avals)
    all_in_names = list(in_names) + list(out_names)
    if partition_name is not None:
        all_in_names.append(partition_name)
    donate = tuple(range(n_params, n_params + n_outs))

    def _body(*args):
        operands = list(args)
        if partition_name is not None:
            operands.append(partition_id_tensor())
        outs = _bass_exec_p.bind(
            *operands,
            out_avals=tuple(out_avals),
            in_names=tuple(all_in_names),
            out_names=tuple(out_names),
            lowering_input_output_aliases=(),
            sim_require_finite=True,
            sim_require_nnan=True,
            nc=nc,
        )
        return tuple(outs)

    devices = jax.devices()[:NCORES]
    mesh = Mesh(np.asarray(devices), ("core",))
    in_specs = (PartitionSpec("core"),) * (n_params + n_outs)
    out_specs = (PartitionSpec("core"),) * n_outs
    sharded = jax.jit(
        shard_map(
            _body, mesh=mesh, in_specs=in_specs, out_specs=out_specs,
            check_rep=False,
        ),
        donate_argnums=donate,
        keep_unused=True,
    )

    def run(in_maps):
        concat_in = [
            np.concatenate([m[name] for m in in_maps], axis=0)
            for name in in_names
        ]
        concat_zeros = [
            np.zeros((NCORES * z.shape[0], *z.shape[1:]), z.dtype)
            for z in zero_outs
        ]
        out_arrs = sharded(*concat_in, *concat_zeros)
        return [
            {
                name: np.asarray(out_arrs[i]).reshape(
                    NCORES, *out_avals[i].shape
                )[c]
                for i, name in enumerate(out_names)
            }
            for c in range(NCORES)
        ]

    _RUNNER_CACHE[key] = run
    return run


def _finish(results):
    """Per-core [128, 2] f32 row-sums of max_j H -> scalar chamfer loss."""
    total = 0.0
    for c in range(NCORES):
        total += np.asarray(results[c]["o"], dtype=np.float64).sum()
    return np.float32(-2.0 * total / (N * B))


def kernel(x, y):
    x = np.asarray(x, dtype=np.float32)
    y = np.asarray(y, dtype=np.float32)
    assert x.shape == (B, N, D) and y.shape == (B, M, D)

    in_maps = _make_in_maps(x, y)
    nc = _get_nc()
    run = _get_runner(nc)
    return _finish(run(in_maps))


# revision 16
# speedup vs baseline: 7.2897x; 1.0120x over previous
"""Chamfer loss kernel for Trainium2 (8 NeuronCores, Bass/Tile).

Problem: x (4, 8192, 3), y (4, 8192, 3) fp32.
  dist[b,i,j] = ||x_bi||^2 + ||y_bj||^2 - 2 x_bi . y_bj
  out = mean_b( mean_i min_j dist + mean_j min_i dist )

Sharding: 8 cores = 4 batches x 2 halves. Core (b, h) computes
  - x->y mins for x rows [h*4096, (h+1)*4096) of batch b vs ALL y[b]
  - y->x mins for y rows [h*4096, (h+1)*4096) of batch b vs ALL x[b]
so each core owns full rows of output; no cross-core reduction needed.

Transfer-minimal formulation (the dispatch wall is dominated by the axon
tunnel: ~90 ms latency floor + ~50 MB/s, so bytes moved matter far more
than device cycles; measured device exec is only ~0.4 ms):
  - The host uploads ONE small fp16 "piece" per tensor half per core
    (default f16w2 variant): rows = [A(3), AL(3), n2h, n2l, ones] where
    A+AL ~ coords.T (2-way f16 split, accurate to ~2^-23) and n2* is the
    2-way split of -||p||^2/2 (computed in f64 on host). Both pieces ride
    in one [18, 4096] f16 input -> 1.15 MiB total upload vs 9.4 MiB for
    pre-built 24-row bf16 operands. (A bf16w3 variant with 3-way bf16
    splits and K=24 is kept for reference; f16w2 measured MORE accurate,
    1.4e-5 vs 3.2e-5 rel err. The ones row is uploaded rather than
    memset because compute-engine ops at unaligned partition offsets
    fail BIR verification; DMA row copies have no partition-alignment
    rule.)
  - Matmul computes H = x.y - (||x||^2+||y||^2)/2 = -dist/2. Folding the
    -1/2 into the norm rows on the host makes EVERY operand row a pure
    byte copy of piece rows, so operand assembly is DMA-only (no
    ACT/DVE work): lhs rows [A,A,AL,n2h,n2l,1,1] and rhs rows
    [A,AL,A,1,1,n2h,n2l] (K=13) pair up to give the 3 retained cross
    products (AL.AL' ~2^-24 dropped) + both norms.
    min_j dist = -2 max_j H.
  - Each core uploads only its OWN halves; full-batch operands are
    reconstructed on device via a pair AllGather (cores {2b, 2b+1}) of
    the raw pieces over NeuronLink. db column order after the gather is
    irrelevant: max over db points is order-agnostic.
  - The drain uses max instead of min (H values cluster just below 0 for
    near neighbors, so the fp16 PSUM->SBUF rounding stays harmless, same
    argument as the min formulation). Per-row maxes are folded and
    row-summed ON DEVICE, so each core fetches back only [128, 2] f32
    (8 KiB total vs 512 KiB).

Drain pipeline per 128-row block (PSUM in [128, 2048] 4-bank groups):
even blocks ACT-copy all 4 groups to fp16 in SBUF and DVE tree-maxes
them; odd blocks DVE-direct-reduce group 0 from PSUM and ACT-copy the
remaining 3 (balances ACT vs DVE element traffic).
"""

import numpy as np
import ml_dtypes

B = 4
N = 8192  # x points per batch
M = 8192  # y points per batch
D = 3
NCORES = 8

QROWS = 4096  # query rows per core (half of a batch's points)
DBN = 8192  # database points scanned per query
PROWS = 13  # bf16w3 piece rows: A(3), AL(3), AL2(3), n2h, n2l, n2l2, ones
PROWS_F16 = 9  # f16w2 piece rows: A(3), AL(3), n2h, n2l, ones
KDIM = 24  # augmented contraction dim (bf16w3; f16w2 uses 13)
BLKP = 128  # query rows per matmul block (PSUM partitions)
FREE = 512  # matmul free size (one PSUM fp32 bank)
G2 = 2048  # PSUM drain group (4 banks)
NBLK = QROWS // BLKP  # 32

_NC_CACHE = {}
_RUNNER_CACHE = {}

# "bf16w3": 13-row bf16 pieces (3-way splits, K=24), separate px/py inputs.
# "f16w2": 9-row f16 pieces (2-way splits, K=13), one merged pxy input +
#          single AllGather; ~30% less upload, ~10x coarser (still ~200x
#          inside the 2e-2 gate) numerics.
VARIANT = "f16w2"


def _build_nc(repeat=1, variant=None):
    from contextlib import ExitStack

    import concourse.tile as tile
    from concourse import bacc, mybir

    variant = VARIANT if variant is None else variant
    bf16 = mybir.dt.bfloat16
    f16 = mybir.dt.float16
    f32 = mybir.dt.float32
    mx = mybir.AluOpType.max
    groups = [[0, 1], [2, 3], [4, 5], [6, 7]]

    nc = bacc.Bacc(
        "TRN2", target_bir_lowering=False, debug=False, num_devices=NCORES
    )
    o = nc.dram_tensor("o", [BLKP, 2], f32, kind="ExternalOutput")

    NEG = -float(np.finfo(np.float32).max)

    with tile.TileContext(nc) as tc, ExitStack() as ctx:
        dram = ctx.enter_context(tc.tile_pool(name="dram", bufs=1, space="DRAM"))
        cpool = ctx.enter_context(tc.tile_pool(name="consts", bufs=1))
        ppool = ctx.enter_context(tc.tile_pool(name="psum", bufs=2, space="PSUM"))
        spool = ctx.enter_context(tc.tile_pool(name="scratch", bufs=3))
        opool = ctx.enter_context(tc.tile_pool(name="outs", bufs=1))

        if variant == "bf16w3":
            kdim = 24
            px = nc.dram_tensor("px", [PROWS, QROWS], bf16, kind="ExternalInput")
            py = nc.dram_tensor("py", [PROWS, QROWS], bf16, kind="ExternalInput")

            # -- exchange raw pieces within each batch pair over NeuronLink.
            # Collectives need DRAM bounce buffers (not I/O tensors directly).
            bx = dram.tile([PROWS, QROWS], bf16, tag="bx")
            by = dram.tile([PROWS, QROWS], bf16, tag="by")
            gx = dram.tile([2 * PROWS, QROWS], bf16, tag="gx")
            gy = dram.tile([2 * PROWS, QROWS], bf16, tag="gy")
            nc.gpsimd.dma_start(bx[:], px[:])
            nc.gpsimd.dma_start(by[:], py[:])
            nc.gpsimd.collective_compute(
                "AllGather",
                mybir.AluOpType.bypass,
                replica_groups=groups,
                ins=[bx.opt()],
                outs=[gx.opt()],
            )
            nc.gpsimd.collective_compute(
                "AllGather",
                mybir.AluOpType.bypass,
                replica_groups=groups,
                ins=[by.opt()],
                outs=[gy.opt()],
            )

            # -- operand assembly: pure DMA row copies.
            # lhs rows [A,A,A, AL,AL, AL2, n2(3), ones(3)] from own piece;
            # rhs rows [A,AL,AL2, A,AL, A, ones(3), n2(3)] per gathered half.
            # Row-k products: A.A + A.AL' + A.AL2' + AL.A' + AL.AL' + AL2.A'
            # + n2_q.1 + 1.n2_d = x.y - (|x|^2+|y|^2)/2 = H = -dist/2.
            lhs_x = cpool.tile([kdim, QROWS], bf16, tag="lhs_x")
            lhs_y = cpool.tile([kdim, QROWS], bf16, tag="lhs_y")
            rhs_x = cpool.tile([kdim, DBN], bf16, tag="rhs_x")
            rhs_y = cpool.tile([kdim, DBN], bf16, tag="rhs_y")

            for lhs, piece in ((lhs_x, px), (lhs_y, py)):
                nc.sync.dma_start(lhs[0:3, :], piece[0:3, :])
                nc.sync.dma_start(lhs[3:6, :], piece[0:3, :])
                nc.sync.dma_start(lhs[6:9, :], piece[0:3, :])
                nc.sync.dma_start(lhs[9:12, :], piece[3:6, :])
                nc.sync.dma_start(lhs[12:15, :], piece[3:6, :])
                nc.sync.dma_start(lhs[15:18, :], piece[6:9, :])
                nc.sync.dma_start(lhs[18:21, :], piece[9:12, :])
                for r in range(3):
                    nc.sync.dma_start(lhs[21 + r : 22 + r, :], piece[12:13, :])
            for rhs, g in ((rhs_x, gx), (rhs_y, gy)):
                for hb in range(2):
                    r0 = hb * PROWS
                    cs = slice(hb * QROWS, (hb + 1) * QROWS)
                    nc.sync.dma_start(rhs[0:9, cs], g[r0 : r0 + 9, :])
                    nc.sync.dma_start(rhs[9:15, cs], g[r0 : r0 + 6, :])
                    nc.sync.dma_start(rhs[15:18, cs], g[r0 : r0 + 3, :])
                    nc.sync.dma_start(rhs[21:24, cs], g[r0 + 9 : r0 + 12, :])
                    for r in range(3):
                        nc.sync.dma_start(
                            rhs[18 + r : 19 + r, cs], g[r0 + 12 : r0 + 13, :]
                        )
        else:  # f16w2
            kdim = 13
            pr = PROWS_F16  # 9: A(3), AL(3), n2h, n2l, one
            pxy = nc.dram_tensor(
                "pxy", [2 * pr, QROWS], f16, kind="ExternalInput"
            )

            bxy = dram.tile([2 * pr, QROWS], f16, tag="bxy")
            gxy = dram.tile([4 * pr, QROWS], f16, tag="gxy")
            nc.gpsimd.dma_start(bxy[:], pxy[:])
            nc.gpsimd.collective_compute(
                "AllGather",
                mybir.AluOpType.bypass,
                replica_groups=groups,
                ins=[bxy.opt()],
                outs=[gxy.opt()],
            )

            # lhs rows [A,A,AL, n2h, n2l, one, one] from own piece;
            # rhs rows [A,AL,A, one, one, n2h, n2l] per gathered half.
            # Row-k products: A.A' + A.AL' + AL.A' + n2_q.1 + 1.n2_d = H.
            lhs_x = cpool.tile([kdim, QROWS], f16, tag="lhs_x")
            lhs_y = cpool.tile([kdim, QROWS], f16, tag="lhs_y")
            rhs_x = cpool.tile([kdim, DBN], f16, tag="rhs_x")
            rhs_y = cpool.tile([kdim, DBN], f16, tag="rhs_y")

            for lhs, r0 in ((lhs_x, 0), (lhs_y, pr)):
                nc.sync.dma_start(lhs[0:3, :], pxy[r0 : r0 + 3, :])
                nc.sync.dma_start(lhs[3:6, :], pxy[r0 : r0 + 3, :])
                nc.sync.dma_start(lhs[6:9, :], pxy[r0 + 3 : r0 + 6, :])
                nc.sync.dma_start(lhs[9:11, :], pxy[r0 + 6 : r0 + 8, :])
                nc.sync.dma_start(lhs[11:12, :], pxy[r0 + 8 : r0 + 9, :])
                nc.sync.dma_start(lhs[12:13, :], pxy[r0 + 8 : r0 + 9, :])
            for rhs, po in ((rhs_x, 0), (rhs_y, pr)):
                for hb in range(2):
                    r0 = hb * 2 * pr + po
                    cs = slice(hb * QROWS, (hb + 1) * QROWS)
                    nc.sync.dma_start(rhs[0:6, cs], gxy[r0 : r0 + 6, :])
                    nc.sync.dma_start(rhs[6:9, cs], gxy[r0 : r0 + 3, :])
                    nc.sync.dma_start(rhs[9:10, cs], gxy[r0 + 8 : r0 + 9, :])
                    nc.sync.dma_start(rhs[10:11, cs], gxy[r0 + 8 : r0 + 9, :])
                    nc.sync.dma_start(rhs[11:13, cs], gxy[r0 + 6 : r0 + 8, :])

        s_out = opool.tile([BLKP, 2], f32, tag="out")

        loop_ctx = tc.For_i(0, repeat, 1) if repeat > 1 else None
        if loop_ctx is not None:
            ctx.enter_context(loop_ctx)

        for col, (lhs, rhs) in enumerate(((lhs_x, rhs_y), (lhs_y, rhs_x))):
            # s_o cols [0:NBLK] = tree maxes, [NBLK:2*NBLK] = direct maxes
            s_o = opool.tile([BLKP, 2 * NBLK], f32, tag=f"so{col}")
            nc.gpsimd.memset(s_o[:], NEG)
            for blk in range(NBLK):
                lhs_blk = lhs[:, blk * BLKP : (blk + 1) * BLKP]

                def fill2(grp):
                    ps = ppool.tile([BLKP, G2], f32, tag="ps2")
                    for t in range(G2 // FREE):
                        c0 = grp * G2 + t * FREE
                        nc.tensor.matmul(
                            ps[:, t * FREE : (t + 1) * FREE],
                            lhs_blk,
                            rhs[:, c0 : c0 + FREE],
                            start=True,
                            stop=True,
                        )
                    return ps

                ngroups = DBN // G2  # 4
                direct = blk % 2 == 1
                g0 = 0
                if direct:
                    ps = fill2(0)
                    nc.vector.tensor_reduce(
                        s_o[:, NBLK + blk : NBLK + blk + 1],
                        ps[:],
                        axis=mybir.AxisListType.X,
                        op=mx,
                    )
                    g0 = 1
                na = ngroups - g0
                S = spool.tile([BLKP, na * G2], f16, tag=f"s16_{na}")
                for grp in range(g0, ngroups):
                    ps = fill2(grp)
                    o0 = (grp - g0) * G2
                    nc.scalar.copy(S[:, o0 : o0 + G2], ps[:])
                if na == 3:
                    # 6144 wide: fold the odd group in with two TTs
                    T1 = spool.tile([BLKP, G2], f16, tag="t6a")
                    nc.vector.tensor_tensor(
                        T1[:], S[:, 0:G2], S[:, G2 : 2 * G2], op=mx
                    )
                    T2 = spool.tile([BLKP, G2], f16, tag="t6b")
                    nc.vector.tensor_tensor(
                        T2[:], T1[:], S[:, 2 * G2 : 3 * G2], op=mx
                    )
                    cur, w = T2, G2
                else:
                    cur, w = S, na * G2
                while w > 1024:
                    nxt = spool.tile([BLKP, w // 2], f16, tag=f"t{w // 2}")
                    nc.vector.tensor_tensor(
                        nxt[:], cur[:, 0 : w // 2], cur[:, w // 2 : w], op=mx
                    )
                    cur, w = nxt, w // 2
                nc.vector.tensor_reduce(
                    s_o[:, blk : blk + 1],
                    cur[:],
                    axis=mybir.AxisListType.X,
                    op=mx,
                )
            # per-row max over (tree, direct) halves, then sum rows' maxes
            fold = spool.tile([BLKP, NBLK], f32, tag=f"fold{col}")
            nc.vector.tensor_tensor(
                fold[:], s_o[:, 0:NBLK], s_o[:, NBLK : 2 * NBLK], op=mx
            )
            nc.vector.tensor_reduce(
                s_out[:, col : col + 1],
                fold[:],
                axis=mybir.AxisListType.X,
                op=mybir.AluOpType.add,
            )
        nc.sync.dma_start(o[:], s_out[:])

    nc.compile()
    return nc


def _get_nc():
    if VARIANT not in _NC_CACHE:
        _NC_CACHE[VARIANT] = _build_nc()
    return _NC_CACHE[VARIANT]


def _split3(a):
    """fp32 array -> (hi, mid, lo) bf16 triple, hi+mid+lo ~ a to ~2^-27 |a|."""
    hi = a.astype(ml_dtypes.bfloat16)
    r = a - hi.astype(np.float32)
    mid = r.astype(ml_dtypes.bfloat16)
    lo = (r - mid.astype(np.float32)).astype(ml_dtypes.bfloat16)
    return hi, mid, lo


def _split2_f16(a):
    """fp32 array -> (hi, lo) f16 pair, hi+lo ~ a to ~2^-23 |a|."""
    hi = a.astype(np.float16)
    lo = (a - hi.astype(np.float32)).astype(np.float16)
    return hi, lo


def _piece(p):
    """p [Q, 3] fp32 -> uploaded piece [13, Q] bf16."""
    P = np.ascontiguousarray(p.T)  # [3, Q]
    A, AL, AL2 = _split3(P)
    h2 = (-0.5 * (p.astype(np.float64) ** 2).sum(axis=1)).astype(np.float32)
    n2h, n2l, n2l2 = _split3(h2[None, :])
    ones = np.ones((1, p.shape[0]), dtype=ml_dtypes.bfloat16)
    return np.concatenate([A, AL, AL2, n2h, n2l, n2l2, ones], axis=0)


def _piece_f16(p):
    """p [Q, 3] fp32 -> uploaded piece [9, Q] f16."""
    P = np.ascontiguousarray(p.T)  # [3, Q]
    A, AL = _split2_f16(P)
    h2 = (-0.5 * (p.astype(np.float64) ** 2).sum(axis=1)).astype(np.float32)
    n2h, n2l = _split2_f16(h2[None, :])
    ones = np.ones((1, p.shape[0]), dtype=np.float16)
    return np.concatenate([A, AL, n2h, n2l, ones], axis=0)


def _pieces_f16(t):
    """t [B, 8192, 3] fp32 -> per-core pieces [8, 9, 4096] f16.

    Core c = (b, h) owns half h of batch b, i.e. row c of t.reshape(8, ...).
    """
    th = t.reshape(B * 2, QROWS, D)
    P = np.ascontiguousarray(th.transpose(0, 2, 1), dtype=np.float32)
    A, AL = _split2_f16(P)
    h2 = (-0.5 * (th.astype(np.float64) ** 2).sum(axis=2)).astype(np.float32)
    n2h, n2l = _split2_f16(h2[:, None, :])
    ones = np.ones((B * 2, 1, QROWS), dtype=np.float16)
    return np.concatenate([A, AL, n2h, n2l, ones], axis=1)


def _make_in_maps(x, y):
    if VARIANT == "bf16w3":
        in_maps = []
        for c in range(NCORES):
            b, h = divmod(c, 2)
            sl = slice(h * QROWS, (h + 1) * QROWS)
            in_maps.append({"px": _piece(x[b, sl]), "py": _piece(y[b, sl])})
        return in_maps
    pxs = _pieces_f16(x)
    pys = _pieces_f16(y)
    return [
        {"pxy": np.concatenate([pxs[c], pys[c]], axis=0)}
        for c in range(NCORES)
    ]


def _get_runner(nc):
    """Build (once) a cached jitted SPMD dispatcher for `nc`.

    Same lowering as concourse.bass_utils.run_bass_kernel_spmd under axon
    (shard_map over 8 cores of a bass_exec custom call), but the jitted
    callable is reused across kernel() invocations, saving the per-call
    retrace/relower (~100 ms).
    """
    key = id(nc)
    if key in _RUNNER_CACHE:
        return _RUNNER_CACHE[key]

    import jax
    import numpy as np
    from jax.sharding import Mesh, PartitionSpec

    try:
        from jax.experimental.shard_map import shard_map
    except ImportError:  # newer jax
        from jax.shard_map import shard_map  # type: ignore

    from concourse import mybir
    from concourse.bass2jax import (
        _bass_exec_p,
        install_neuronx_cc_hook,
        partition_id_tensor,
    )

    install_neuronx_cc_hook()

    partition_name = (
        nc.partition_id_tensor.name if nc.partition_id_tensor else None
    )
    in_names = []
    out_names = []
    out_avals = []
    zero_outs = []
    for alloc in nc.m.functions[0].allocations:
        if not isinstance(alloc, mybir.MemoryLocationSet):
            continue
        name = alloc.memorylocations[0].name
        if alloc.kind == "ExternalInput":
            if name != partition_name:
                in_names.append(name)
        elif alloc.kind == "ExternalOutput":
            shape = tuple(alloc.tensor_shape)
            dtype = mybir.dt.np(alloc.dtype)
            out_names.append(name)
            out_avals.append(jax.core.ShapedArray(shape, dtype))
            zero_outs.append(np.zeros(shape, dtype))
    n_params = len(in_names)
    n_outs = len(out_avals)
    all_in_names = list(in_names) + list(out_names)
    if partition_name is not None:
        all_in_names.append(partition_name)
    donate = tuple(range(n_params, n_params + n_outs))

    def _body(*args):
        operands = list(args)
        if partition_name is not None:
            operands.append(partition_id_tensor())
        outs = _bass_exec_p.bind(
            *operands,
            out_avals=tuple(out_avals),
            in_names=tuple(all_in_names),
            out_names=tuple(out_names),
            lowering_input_output_aliases=(),
            sim_require_finite=True,
            sim_require_nnan=True,
            nc=nc,
        )
        return tuple(outs)

    devices = jax.devices()[:NCORES]
    assert len(devices) == NCORES and devices[0].platform != "cpu", (
        f"need {NCORES} accelerator devices, got {jax.devices()}"
    )
    mesh = Mesh(np.asarray(devices), ("core",))
    in_specs = (PartitionSpec("core"),) * (n_params + n_outs)
    out_specs = (PartitionSpec("core"),) * n_outs
    sharded = jax.jit(
        shard_map(
            _body, mesh=mesh, in_specs=in_specs, out_specs=out_specs,
            check_rep=False,
        ),
        donate_argnums=donate,
        keep_unused=True,
    )

    def run(in_maps):
        concat_in = [
            np.concatenate([m[name] for m in in_maps], axis=0)
            for name in in_names
        ]
        concat_zeros = [
            np.zeros((NCORES * z.shape[0], *z.shape[1:]), z.dtype)
            for z in zero_outs
        ]
        out_arrs = sharded(*concat_in, *concat_zeros)
        return [
            {
                name: np.asarray(out_arrs[i]).reshape(
                    NCORES, *out_avals[i].shape
                )[c]
                for i, name in enumerate(out_names)
            }
            for c in range(NCORES)
        ]

    _RUNNER_CACHE[key] = run
    return run


def _finish(results):
    """Per-core [128, 2] f32 row-sums of max_j H -> scalar chamfer loss."""
    total = 0.0
    for c in range(NCORES):
        total += np.asarray(results[c]["o"], dtype=np.float64).sum()
    return np.float32(-2.0 * total / (N * B))


def kernel(x, y):
    x = np.asarray(x, dtype=np.float32)
    y = np.asarray(y, dtype=np.float32)
    assert x.shape == (B, N, D) and y.shape == (B, M, D)

    in_maps = _make_in_maps(x, y)
    nc = _get_nc()
    try:
        run = _get_runner(nc)
        results = run(in_maps)
    except Exception:
        # Fall back to the stock dispatcher (also covers native-NRT
        # environments where the cached PJRT runner path doesn't apply).
        from concourse.bass_utils import run_bass_kernel_spmd

        results = run_bass_kernel_spmd(
            nc, in_maps, core_ids=list(range(NCORES))
        ).results
    return _finish(results)
